# revision 8
# baseline (speedup 1.0000x reference)
"""EulerGCN on 8 trn2 NeuronCores — two SPMD launches.

K1: core t owns snapshot t. table1 = dinv*(x@W1) on device; 2 GCN props
    via ELL gathers + DVE tree reduce + unique-idx scatter-add into
    natural-order accumulators; relu/W2/tanh finish. Output tanhE [NPAD,H].
K2: node-sharded GRU + linear head.

Host does integer layout (edge grouping, degree sort, idx grids) and the
standard GCN normalization coefficients dinv = rsqrt(deg).
"""

import numpy as np
import concourse.bass as bass
import concourse.bacc as bacc
import concourse.mybir as mybir
import concourse.tile as tile
from concourse.bass_utils import run_bass_kernel_spmd
from concourse.masks import make_identity

phase_times = {}
last_exec_time_ns = None

P = 128
NCORES = 8
N = 100000
NPAD = 100352           # 784 blocks of 128; 4 quarters of 25088
QN = NPAD // 4          # 25088
NBLK = NPAD // P        # 784
T = 8
XD = 128
H = 64
Z = 32
ACCR = QN + P           # accumulator rows per quarter (+dummy block)
F32 = mybir.dt.float32
I16 = mybir.dt.int16
NSH = NPAD // NCORES    # 12544 nodes per core in K2
JC = NSH // P           # 98 columns


def wrap16(a):
    n = a.shape[0]
    return np.ascontiguousarray(np.tile(a.reshape(n // 16, 16).T, (8, 1)))


def build_structure(ei, ew, dinv):
    """Per-snapshot host structure: 16 (dst-quarter r, src-quarter q)
    sections; nodes degree-sorted per section; returns per-section
    per-block grids as streams."""
    src = ei[0].astype(np.int64)
    dst = ei[1].astype(np.int64)
    w = (ew * dinv[dst]).astype(np.float32)  # dinv[src] folded into table
    out = []
    for r in range(4):
        for q in range(4):
            m = (dst // QN == r) & (src // QN == q)
            s_src = src[m] - q * QN
            s_dst = dst[m] - r * QN
            s_w = w[m]
            cnt = np.bincount(s_dst, minlength=QN)
            order = np.argsort(-cnt, kind="stable")
            nactive = int((cnt > 0).sum())
            nact_pad = max(P, ((nactive + P - 1) // P) * P)
            nodes = order[:nact_pad]
            nblocks = nact_pad // P
            counts = cnt[nodes].astype(np.int64)
            Ls = counts.reshape(nblocks, P).max(axis=1).astype(np.int32)
            rank_of = np.full(QN, -1, np.int64)
            rank_of[nodes] = np.arange(nact_pad)
            erank = rank_of[s_dst]
            eorder = np.argsort(erank, kind="stable")
            er_sorted = erank[eorder]
            slot = np.arange(er_sorted.size) - np.searchsorted(er_sorted, er_sorted)
            out.append({
                "nodes": nodes, "Ls": Ls, "nblocks": nblocks,
                "e_src": s_src[eorder], "e_w": s_w[eorder],
                "e_rank": er_sorted, "e_slot": slot,
            })
    return out


def kernel(**inputs):
    import time as _time
    _t0 = _time.time()
    x = np.asarray(inputs["x"], np.float32)
    eis = np.asarray(inputs["eis"])
    ews = np.asarray(inputs["ews"], np.float32)
    W1 = np.asarray(inputs["W1"], np.float32)
    b1 = np.asarray(inputs["b1"], np.float32)
    W2 = np.asarray(inputs["W2"], np.float32)
    b2 = np.asarray(inputs["b2"], np.float32)
    Wih = np.asarray(inputs["Wih"], np.float32)
    Whh = np.asarray(inputs["Whh"], np.float32)
    bih = np.asarray(inputs["bih"], np.float32)
    bhh = np.asarray(inputs["bhh"], np.float32)
    Wlin = np.asarray(inputs["Wlin"], np.float32)
    blin = np.asarray(inputs["blin"], np.float32)

    # ---------------- host layout ----------------
    dinvs = []
    for t in range(T):
        deg = np.bincount(eis[t, 1].astype(np.int64), weights=ews[t],
                          minlength=N).astype(np.float32) + 1.0
        dinv = (1.0 / np.sqrt(deg)).astype(np.float32)
        dinvs.append(np.concatenate([dinv, np.zeros(NPAD - N, np.float32)]))
    structs = [build_structure(eis[t], ews[t], dinvs[t][:N]) for t in range(T)]

    # common per-section per-block L (max over cores; degree-sorted so tight)
    commonL = []
    for s in range(16):
        nb = max(st[s]["nblocks"] for st in structs)
        Lc = np.zeros(nb, np.int32)
        for st in structs:
            Ls = st[s]["Ls"]
            Lc[:len(Ls)] = np.maximum(Lc[:len(Ls)], Ls)
        commonL.append(Lc)

    tok_per_sec = [int(L.sum()) * P for L in commonL]
    blk_per_sec = [len(L) for L in commonL]
    tok_total = sum(tok_per_sec)

    per_gidx, per_w, per_sidx = [], [], []
    for c in range(T):
        g_all, w_all, s_all = [], [], []
        for s in range(16):
            sec = structs[c][s]
            Lc = commonL[s]
            nbm = len(Lc)
            own_nb = sec["nblocks"]
            # per-block dense grids in common shape
            for b in range(nbm):
                Lb = int(Lc[b])
                gi = np.zeros((Lb, P), np.int64)
                gw = np.zeros((Lb, P), np.float32)
                if b < own_nb:
                    sel = (sec["e_rank"] >= b * P) & (sec["e_rank"] < (b + 1) * P)
                    rr = sec["e_rank"][sel] - b * P
                    ss = sec["e_slot"][sel]
                    gi[ss, rr] = sec["e_src"][sel]
                    gw[ss, rr] = sec["e_w"][sel]
                g_all.append(gi.reshape(-1))
                w_all.append(gw.reshape(-1))
                if b < own_nb:
                    nd = sec["nodes"][b * P:(b + 1) * P].astype(np.int64)
                else:
                    nd = np.arange(QN, QN + P, dtype=np.int64)  # dummy block
                s_all.append(nd)
        per_gidx.append(np.concatenate(g_all).astype(np.int16))
        per_w.append(np.concatenate(w_all).astype(np.float32))
        per_sidx.append(np.concatenate(s_all).astype(np.int16))

    scat_total = sum(blk_per_sec) * P

    # blocked dinv layouts [128, NBLK]: col k = nodes k*128+p
    dinv_blk = [d.reshape(NBLK, P).T.copy() for d in dinvs]
    dinv2_blk = [(d * d).reshape(NBLK, P).T.copy() for d in dinvs]

    x_pad = np.zeros((NPAD, XD), np.float32)
    x_pad[:N] = x
    b1b = np.broadcast_to(b1, (P, H)).copy()
    b2b = np.broadcast_to(b2, (P, H)).copy()

    phase_times["host_layout"] = _time.time() - _t0
    _t0 = _time.time()

    # ---------------- K1 program ----------------
    nc1 = bacc.Bacc(trn_type="TRN2", num_devices=NCORES, num_swdge_queues=4)
    x_d = nc1.dram_tensor("x", [NPAD, XD], F32, kind="ExternalInput")
    W1_d = nc1.dram_tensor("W1", [XD, H], F32, kind="ExternalInput")
    W2_d = nc1.dram_tensor("W2", [H, H], F32, kind="ExternalInput")
    b1_d = nc1.dram_tensor("b1b", [P, H], F32, kind="ExternalInput")
    b2_d = nc1.dram_tensor("b2b", [P, H], F32, kind="ExternalInput")
    dinv_d = nc1.dram_tensor("dinv_blk", [P, NBLK], F32, kind="ExternalInput")
    dinv2_d = nc1.dram_tensor("dinv2_blk", [P, NBLK], F32, kind="ExternalInput")
    gidx_d = nc1.dram_tensor("gidx", [P, tok_total // 16], I16, kind="ExternalInput")
    gw_d = nc1.dram_tensor("gw", [P, tok_total // P], F32, kind="ExternalInput")
    sidx_d = nc1.dram_tensor("sidx", [P, scat_total // 16], I16, kind="ExternalInput")
    tanhE_d = nc1.dram_tensor("tanhE", [NPAD, H], F32, kind="ExternalOutput")

    table1 = nc1.dram_tensor("table1", [NPAD, H], F32)
    table2 = nc1.dram_tensor("table2", [NPAD, H], F32)
    acc = [nc1.dram_tensor(f"acc{pr}", [4 * ACCR, H], F32) for pr in range(2)]

    with tile.TileContext(nc1) as tc:
        with (
            tc.tile_pool(name="const", bufs=1) as cpool,
            tc.tile_pool(name="work", bufs=3) as wpool,
            tc.tile_pool(name="gath", bufs=3) as gpool,
            tc.tile_pool(name="psum", bufs=2, space="PSUM") as ppool,
        ):
            ident = cpool.tile([P, P], F32)
            make_identity(nc1, ident[:])
            W1_t = cpool.tile([XD, H], F32)
            W2_t = cpool.tile([H, H], F32)
            b1_t = cpool.tile([P, H], F32)
            b2_t = cpool.tile([P, H], F32)
            dinv_t = cpool.tile([P, NBLK], F32)
            dinv2_t = cpool.tile([P, NBLK], F32)
            nc1.sync.dma_start(out=W1_t[:], in_=W1_d[:])
            nc1.sync.dma_start(out=W2_t[:], in_=W2_d[:])
            nc1.sync.dma_start(out=b1_t[:], in_=b1_d[:])
            nc1.sync.dma_start(out=b2_t[:], in_=b2_d[:])
            nc1.sync.dma_start(out=dinv_t[:], in_=dinv_d[:])
            nc1.sync.dma_start(out=dinv2_t[:], in_=dinv2_d[:])

            # zero accumulators
            zt = cpool.tile([P, 512], F32)
            nc1.gpsimd.memset(zt[:], 0.0)
            for pr in range(2):
                rows = 4 * ACCR
                a0 = 0
                while a0 < rows:
                    a1 = min(a0 + 1024, rows)
                    nc1.sync.dma_start(out=acc[pr][a0:a1, :],
                                       in_=zt[:, :(a1 - a0) // 2])
                    a0 = a1

            # stage A: table1 = dinv * (x @ W1)
            for k in range(NBLK):
                xb = wpool.tile([P, XD], F32, tag="xb")
                nc1.sync.dma_start(out=xb[:], in_=x_d[k * P:(k + 1) * P, :])
                xT_p = ppool.tile([P, P], F32, tag="pt")
                nc1.tensor.transpose(out=xT_p[:], in_=xb[:], identity=ident[:])
                xT = wpool.tile([P, P], F32, tag="xT")
                nc1.vector.tensor_copy(out=xT[:], in_=xT_p[:])
                mm = ppool.tile([P, H], F32, tag="mm")
                nc1.tensor.matmul(out=mm[:], lhsT=xT[:], rhs=W1_t[:],
                                  start=True, stop=True)
                tb = wpool.tile([P, H], F32, tag="tb")
                nc1.vector.tensor_scalar_mul(
                    out=tb[:], in0=mm[:], scalar1=dinv_t[:, k:k + 1])
                nc1.sync.dma_start(out=table1[k * P:(k + 1) * P, :], in_=tb[:])

            # the two props
            for pr in range(2):
                table = table1 if pr == 0 else table2
                go = 0   # token offset
                so = 0   # scatter token offset
                for s in range(16):
                    r, q = divmod(s, 4)
                    Lc = commonL[s]
                    tbl_slice = table[q * QN:(q + 1) * QN, :]
                    b = 0
                    while b < len(Lc):
                        L = int(Lc[b])
                        b2_ = b
                        while b2_ < len(Lc) and int(Lc[b2_]) == L:
                            b2_ += 1
                        nb = b2_ - b
                        if L == 0:
                            b = b2_
                            continue
                        # chunk over blocks (and slots if L>64)
                        gpc = max(1, 64 // L) if L <= 64 else 1
                        sub = min(L, 64)
                        bb = b
                        while bb < b2_:
                            nbb = min(gpc, b2_ - bb)
                            if L <= 64:
                                ncols = nbb * L
                                tok = ncols * P
                                gt = gpool.tile([P, 64, H], F32, tag="g")
                                gi_t = gpool.tile([P, 512], I16, tag="gi")
                                w_t = gpool.tile([P, 64], F32, tag="gwt")
                                nc1.sync.dma_start(
                                    out=gi_t[:, :tok // 16],
                                    in_=gidx_d[:, go // 16:(go + tok) // 16])
                                nc1.sync.dma_start(
                                    out=w_t[:, :ncols],
                                    in_=gw_d[:, go // P:(go + tok) // P])
                                nc1.gpsimd.dma_gather(
                                    out_ap=gt[:, :ncols, :], in_ap=tbl_slice,
                                    idxs_ap=gi_t[:, :tok // 16],
                                    num_idxs=tok, num_idxs_reg=tok,
                                    elem_size=H, single_packet=False,
                                    queue_num=(bb + s) % 4)
                                nc1.vector.tensor_tensor(
                                    out=gt[:, :ncols, :], in0=gt[:, :ncols, :],
                                    in1=w_t[:, :ncols].to_broadcast([P, ncols, H]),
                                    op=mybir.AluOpType.mult)
                                # tree reduce per g-group: view [P, nbb, L, H]
                                gv = gt[:, :ncols, :].rearrange(
                                    "p (g l) h -> p g l h", l=L)
                                width = L
                                while width > 1:
                                    half = width // 2
                                    nc1.vector.tensor_tensor(
                                        out=gv[:, :, :half, :],
                                        in0=gv[:, :, :half, :],
                                        in1=gv[:, :, width - half:width, :],
                                        op=mybir.AluOpType.add)
                                    width = width - half
                                # pack partials [P, nbb, H]
                                pk = gpool.tile([P, 64, H], F32, tag="pk")
                                nc1.vector.tensor_copy(
                                    out=pk[:, :nbb, :], in_=gv[:, :, 0, :])
                                stok_all = nbb * P
                                si_t = gpool.tile([P, 512], I16, tag="si")
                                nc1.sync.dma_start(
                                    out=si_t[:, :stok_all // 16],
                                    in_=sidx_d[:, so // 16:(so + stok_all) // 16])
                                for c0 in range(0, nbb, 8):
                                    c1 = min(c0 + 8, nbb)
                                    stok = (c1 - c0) * P
                                    nc1.gpsimd.dma_scatter_add(
                                        acc[pr][r * ACCR:(r + 1) * ACCR, :],
                                        pk[:, c0:c1, :],
                                        si_t[:, c0 * 8:c0 * 8 + stok // 16],
                                        stok, stok, H)
                                so += stok_all
                                go += tok
                            else:
                                # L > 64: one block, slot sub-chunks
                                first = True
                                for s0 in range(0, L, sub):
                                    s1 = min(s0 + sub, L)
                                    ncols = s1 - s0
                                    tok = ncols * P
                                    gt = gpool.tile([P, 64, H], F32, tag="g")
                                    gi_t = gpool.tile([P, 512], I16, tag="gi")
                                    w_t = gpool.tile([P, 64], F32, tag="gwt")
                                    nc1.sync.dma_start(
                                        out=gi_t[:, :tok // 16],
                                        in_=gidx_d[:, go // 16:(go + tok) // 16])
                                    nc1.sync.dma_start(
                                        out=w_t[:, :ncols],
                                        in_=gw_d[:, go // P:(go + tok) // P])
                                    nc1.gpsimd.dma_gather(
                                        out_ap=gt[:, :ncols, :], in_ap=tbl_slice,
                                        idxs_ap=gi_t[:, :tok // 16],
                                        num_idxs=tok, num_idxs_reg=tok,
                                        elem_size=H, single_packet=False,
                                        queue_num=(bb + s0 + s) % 4)
                                    nc1.vector.tensor_tensor(
                                        out=gt[:, :ncols, :], in0=gt[:, :ncols, :],
                                        in1=w_t[:, :ncols].to_broadcast([P, ncols, H]),
                                        op=mybir.AluOpType.mult)
                                    width = ncols
                                    gv = gt[:, :ncols, :].rearrange(
                                        "p (g l) h -> p g l h", l=ncols)
                                    while width > 1:
                                        half = width // 2
                                        nc1.vector.tensor_tensor(
                                            out=gv[:, :, :half, :],
                                            in0=gv[:, :, :half, :],
                                            in1=gv[:, :, width - half:width, :],
                                            op=mybir.AluOpType.add)
                                        width = width - half
                                    pk = gpool.tile([P, 64, H], F32, tag="pk")
                                    nc1.vector.tensor_copy(
                                        out=pk[:, :1, :], in_=gv[:, :, 0, :])
                                    si_t = gpool.tile([P, 64], I16, tag="si")
                                    nc1.sync.dma_start(
                                        out=si_t[:, :P // 16],
                                        in_=sidx_d[:, so // 16:(so + P) // 16])
                                    nc1.gpsimd.dma_scatter_add(
                                        acc[pr][r * ACCR:(r + 1) * ACCR, :],
                                        pk[:, :1, :], si_t[:, :P // 16],
                                        P, P, H)
                                    go += tok
                                    first = False
                                so += P
                            bb += nbb if L <= 64 else 1
                        b = b2_

                # finish pass
                for k in range(NBLK):
                    r = (k * P) // QN
                    lrow = (k * P) % QN
                    ab = wpool.tile([P, H], F32, tag="ab")
                    nc1.sync.dma_start(
                        out=ab[:], in_=acc[pr][r * ACCR + lrow:r * ACCR + lrow + P, :])
                    tb = wpool.tile([P, H], F32, tag="tb2")
                    nc1.sync.dma_start(out=tb[:], in_=table[k * P:(k + 1) * P, :])
                    init = wpool.tile([P, H], F32, tag="init")
                    nc1.vector.tensor_scalar_mul(
                        out=init[:], in0=tb[:], scalar1=dinv_t[:, k:k + 1])
                    nc1.vector.tensor_tensor(out=ab[:], in0=ab[:], in1=init[:],
                                             op=mybir.AluOpType.add)
                    if pr == 0:
                        # h1 = relu(ab + b1); table2 = dinv * h1
                        nc1.vector.tensor_tensor(out=ab[:], in0=ab[:], in1=b1_t[:],
                                                 op=mybir.AluOpType.add)
                        nc1.vector.tensor_scalar_max(out=ab[:], in0=ab[:], scalar1=0.0)
                        ot = wpool.tile([P, H], F32, tag="ot")
                        nc1.vector.tensor_scalar_mul(
                            out=ot[:], in0=ab[:], scalar1=dinv_t[:, k:k + 1])
                        nc1.sync.dma_start(out=table2[k * P:(k + 1) * P, :], in_=ot[:])
                    else:
                        # embeds = ab @ W2 + b2 ; tanhE = tanh(embeds)
                        tp = ppool.tile([P, P], F32, tag="pt")
                        nc1.tensor.transpose(out=tp[:H, :], in_=ab[:],
                                             identity=ident[:])
                        abT = wpool.tile([P, P], F32, tag="abT")
                        nc1.vector.tensor_copy(out=abT[:H, :], in_=tp[:H, :])
                        mm = ppool.tile([P, H], F32, tag="mm")
                        nc1.tensor.matmul(out=mm[:], lhsT=abT[:H, :], rhs=W2_t[:],
                                          start=True, stop=True)
                        eb = wpool.tile([P, H], F32, tag="eb")
                        nc1.vector.tensor_tensor(out=eb[:], in0=mm[:], in1=b2_t[:],
                                                 op=mybir.AluOpType.add)
                        th = wpool.tile([P, H], F32, tag="th")
                        nc1.scalar.activation(
                            out=th[:], in_=eb[:],
                            func=mybir.ActivationFunctionType.Tanh)
                        nc1.sync.dma_start(out=tanhE_d[k * P:(k + 1) * P, :],
                                           in_=th[:])
    nc1.compile()
    phase_times["build_k1"] = _time.time() - _t0
    _t0 = _time.time()

    in_maps1 = []
    for c in range(NCORES):
        in_maps1.append({
            "x": x_pad, "W1": W1, "W2": W2, "b1b": b1b, "b2b": b2b,
            "dinv_blk": dinv_blk[c], "dinv2_blk": dinv2_blk[c],
            "gidx": wrap16(per_gidx[c]),
            "gw": per_w[c].reshape(-1, P).T.copy(),
            "sidx": wrap16(per_sidx[c]),
            "tanhE": np.zeros((NPAD, H), np.float32),
        })
    res1 = run_bass_kernel_spmd(nc1, in_maps1, core_ids=list(range(NCORES)))
    phase_times["run_k1"] = _time.time() - _t0
    _t0 = _time.time()
    tanhE = np.stack([res1.results[c]["tanhE"] for c in range(NCORES)])  # [T,NPAD,H]

    # ---------------- K2: GRU + head, node-sharded ----------------
    WihT = Wih.T.copy()    # [H, 3H]
    WhhT = Whh.T.copy()
    WlinT = Wlin.T.copy()  # [H, Z]
    bihb = np.broadcast_to(bih, (P, 3 * H)).copy()
    bhhb = np.broadcast_to(bhh, (P, 3 * H)).copy()
    blinb = np.broadcast_to(blin, (P, Z)).copy()

    nc2 = bacc.Bacc(trn_type="TRN2", num_devices=NCORES, num_swdge_queues=1)
    xs_d = nc2.dram_tensor("xs", [T, NSH, H], F32, kind="ExternalInput")
    WihT_d = nc2.dram_tensor("WihT", [H, 3 * H], F32, kind="ExternalInput")
    WhhT_d = nc2.dram_tensor("WhhT", [H, 3 * H], F32, kind="ExternalInput")
    WlinT_d = nc2.dram_tensor("WlinT", [H, Z], F32, kind="ExternalInput")
    bih_d = nc2.dram_tensor("bihb", [P, 3 * H], F32, kind="ExternalInput")
    bhh_d = nc2.dram_tensor("bhhb", [P, 3 * H], F32, kind="ExternalInput")
    blin_d = nc2.dram_tensor("blinb", [P, Z], F32, kind="ExternalInput")
    ys_d = nc2.dram_tensor("ys", [T, NSH, Z], F32, kind="ExternalOutput")

    with tile.TileContext(nc2) as tc:
        with (
            tc.tile_pool(name="const", bufs=1) as cpool,
            tc.tile_pool(name="state", bufs=1) as spool,
            tc.tile_pool(name="work", bufs=3) as wpool,
            tc.tile_pool(name="psum", bufs=2, space="PSUM") as ppool,
        ):
            ident = cpool.tile([P, P], F32)
            make_identity(nc2, ident[:])
            WihT_t = cpool.tile([H, 3 * H], F32)
            WhhT_t = cpool.tile([H, 3 * H], F32)
            WlinT_t = cpool.tile([H, Z], F32)
            bih_t = cpool.tile([P, 3 * H], F32)
            bhh_t = cpool.tile([P, 3 * H], F32)
            blin_t = cpool.tile([P, Z], F32)
            for tt, dd in ((WihT_t, WihT_d), (WhhT_t, WhhT_d), (WlinT_t, WlinT_d),
                           (bih_t, bih_d), (bhh_t, bhh_d), (blin_t, blin_d)):
                nc2.sync.dma_start(out=tt[:], in_=dd[:])

            JC2 = JC // 2
            NSH2 = NSH // 2
            for half in range(2):
                h_t = spool.tile([P, JC2, H], F32, tag="h")
                nc2.gpsimd.memset(h_t[:], 0.0)
                for t in range(T):
                    xs_t = spool.tile([P, JC2, H], F32, tag="xs")
                    xv = xs_d[t][half * NSH2:(half + 1) * NSH2, :].rearrange(
                        "(p j) h -> p j h", j=JC2)
                    nc2.sync.dma_start(out=xs_t[:], in_=xv)
                    gi_t = spool.tile([P, JC2, 3 * H], F32, tag="gi")
                    gh_t = spool.tile([P, JC2, 3 * H], F32, tag="gh")
                    for j in range(JC2):
                        for which in range(2):
                            srcT = xs_t if which == 0 else h_t
                            dstT = gi_t if which == 0 else gh_t
                            tp = ppool.tile([P, P], F32, tag="pt")
                            nc2.tensor.transpose(out=tp[:H, :], in_=srcT[:, j, :],
                                                 identity=ident[:])
                            sT = wpool.tile([H, P], F32, tag="sT")
                            nc2.vector.tensor_copy(out=sT[:], in_=tp[:H, :])
                            mm = ppool.tile([P, 3 * H], F32, tag="mm")
                            WT = WihT_t if which == 0 else WhhT_t
                            nc2.tensor.matmul(out=mm[:], lhsT=sT[:], rhs=WT[:],
                                              start=True, stop=True)
                            bT = bih_t if which == 0 else bhh_t
                            nc2.vector.tensor_tensor(out=dstT[:, j, :], in0=mm[:],
                                                     in1=bT[:],
                                                     op=mybir.AluOpType.add)
                    rz = spool.tile([P, JC2, 2 * H], F32, tag="rz")
                    nc2.vector.tensor_tensor(out=rz[:], in0=gi_t[:, :, :2 * H],
                                             in1=gh_t[:, :, :2 * H],
                                             op=mybir.AluOpType.add)
                    nc2.scalar.activation(out=rz[:], in_=rz[:],
                                          func=mybir.ActivationFunctionType.Sigmoid)
                    nn_t = spool.tile([P, JC2, H], F32, tag="nn")
                    nc2.vector.tensor_tensor(out=nn_t[:], in0=rz[:, :, :H],
                                             in1=gh_t[:, :, 2 * H:],
                                             op=mybir.AluOpType.mult)
                    nc2.vector.tensor_tensor(out=nn_t[:], in0=nn_t[:],
                                             in1=gi_t[:, :, 2 * H:],
                                             op=mybir.AluOpType.add)
                    nc2.scalar.activation(out=nn_t[:], in_=nn_t[:],
                                          func=mybir.ActivationFunctionType.Tanh)
                    dz = spool.tile([P, JC2, H], F32, tag="dz")
                    nc2.vector.tensor_tensor(out=dz[:], in0=h_t[:], in1=nn_t[:],
                                             op=mybir.AluOpType.subtract)
                    nc2.vector.tensor_tensor(out=dz[:], in0=dz[:], in1=rz[:, :, H:],
                                             op=mybir.AluOpType.mult)
                    nc2.vector.tensor_tensor(out=h_t[:], in0=nn_t[:], in1=dz[:],
                                             op=mybir.AluOpType.add)
                    ys_t = spool.tile([P, JC2, Z], F32, tag="ys")
                    for j in range(JC2):
                        tp = ppool.tile([P, P], F32, tag="pt")
                        nc2.tensor.transpose(out=tp[:H, :], in_=h_t[:, j, :],
                                             identity=ident[:])
                        sT = wpool.tile([H, P], F32, tag="sT")
                        nc2.vector.tensor_copy(out=sT[:], in_=tp[:H, :])
                        mm = ppool.tile([P, Z], F32, tag="mmz")
                        nc2.tensor.matmul(out=mm[:], lhsT=sT[:], rhs=WlinT_t[:],
                                          start=True, stop=True)
                        nc2.vector.tensor_tensor(out=ys_t[:, j, :], in0=mm[:],
                                                 in1=blin_t[:],
                                                 op=mybir.AluOpType.add)
                    yv = ys_d[t][half * NSH2:(half + 1) * NSH2, :].rearrange(
                        "(p j) z -> p j z", j=JC2)
                    nc2.sync.dma_start(out=yv, in_=ys_t[:])
    nc2.compile()
    phase_times["build_k2"] = _time.time() - _t0
    _t0 = _time.time()

    in_maps2 = []
    for c in range(NCORES):
        xs = np.ascontiguousarray(tanhE[:, c * NSH:(c + 1) * NSH, :])
        in_maps2.append({
            "xs": xs, "WihT": WihT, "WhhT": WhhT, "WlinT": WlinT,
            "bihb": bihb, "bhhb": bhhb, "blinb": blinb,
            "ys": np.zeros((T, NSH, Z), np.float32),
        })
    res2 = run_bass_kernel_spmd(nc2, in_maps2, core_ids=list(range(NCORES)))
    phase_times["run_k2"] = _time.time() - _t0
    out = np.concatenate([res2.results[c]["ys"] for c in range(NCORES)], axis=1)
    return np.ascontiguousarray(out[:, :N, :])



# revision 29
# speedup vs baseline: 1.4389x; 1.4389x over previous
"""EulerGCN on 8 trn2 NeuronCores — two SPMD launches.

K1: core t owns snapshot t. table1 = dinv*(x@W1) on device; 2 GCN props
    via ELL gathers + DVE tree reduce + unique-idx scatter-add into
    natural-order accumulators; relu/W2/tanh finish. Output tanhE [NPAD,H].
K2: node-sharded GRU + linear head.

Host does integer layout (edge grouping, degree sort, idx grids) and the
standard GCN normalization coefficients dinv = rsqrt(deg).
"""

import numpy as np
import concourse.bass as bass
import concourse.bacc as bacc
import concourse.mybir as mybir
import concourse.tile as tile
from concourse.bass_utils import run_bass_kernel_spmd
from concourse.masks import make_identity

phase_times = {}
last_exec_time_ns = None

P = 128
NCORES = 8
N = 100000
NPAD = 100352           # 784 blocks of 128; 4 quarters of 25088
QN = NPAD // 4          # 25088
NBLK = NPAD // P        # 784
T = 8
XD = 128
H = 64
Z = 32
ACCR = QN + P           # accumulator rows per quarter (+dummy block)
F32 = mybir.dt.float32
I16 = mybir.dt.int16
NSH = NPAD // NCORES    # 12544 nodes per core in K2
JC = NSH // P           # 98 columns


def wrap16(a):
    n = a.shape[0]
    return np.ascontiguousarray(np.tile(a.reshape(n // 16, 16).T, (8, 1)))


def build_structure(ei, ew, dinv):
    """Per-snapshot host structure: 16 (dst-quarter r, src-quarter q)
    sections; nodes degree-sorted per section; returns per-section
    per-block grids as streams."""
    src = ei[0].astype(np.int64)
    dst = ei[1].astype(np.int64)
    w = (ew * dinv[dst]).astype(np.float32)  # dinv[src] folded into table
    out = []
    for r in range(4):
        for q in range(4):
            m = (dst // QN == r) & (src // QN == q)
            s_src = src[m] - q * QN
            s_dst = dst[m] - r * QN
            s_w = w[m]
            cnt = np.bincount(s_dst, minlength=QN)
            order = np.argsort(-cnt, kind="stable")
            nactive = int((cnt > 0).sum())
            nact_pad = max(P, ((nactive + P - 1) // P) * P)
            nodes = order[:nact_pad]
            nblocks = nact_pad // P
            counts = cnt[nodes].astype(np.int64)
            Ls = counts.reshape(nblocks, P).max(axis=1).astype(np.int32)
            rank_of = np.full(QN, -1, np.int64)
            rank_of[nodes] = np.arange(nact_pad)
            erank = rank_of[s_dst]
            eorder = np.argsort(erank, kind="stable")
            er_sorted = erank[eorder]
            slot = np.arange(er_sorted.size) - np.searchsorted(er_sorted, er_sorted)
            out.append({
                "nodes": nodes, "Ls": Ls, "nblocks": nblocks,
                "e_src": s_src[eorder], "e_w": s_w[eorder],
                "e_rank": er_sorted, "e_slot": slot,
            })
    return out


def kernel(**inputs):
    import time as _time
    _t0 = _time.time()
    x = np.asarray(inputs["x"], np.float32)
    eis = np.asarray(inputs["eis"])
    ews = np.asarray(inputs["ews"], np.float32)
    W1 = np.asarray(inputs["W1"], np.float32)
    b1 = np.asarray(inputs["b1"], np.float32)
    W2 = np.asarray(inputs["W2"], np.float32)
    b2 = np.asarray(inputs["b2"], np.float32)
    Wih = np.asarray(inputs["Wih"], np.float32)
    Whh = np.asarray(inputs["Whh"], np.float32)
    bih = np.asarray(inputs["bih"], np.float32)
    bhh = np.asarray(inputs["bhh"], np.float32)
    Wlin = np.asarray(inputs["Wlin"], np.float32)
    blin = np.asarray(inputs["blin"], np.float32)

    # ---------------- host layout ----------------
    dinvs = []
    for t in range(T):
        deg = np.bincount(eis[t, 1].astype(np.int64), weights=ews[t],
                          minlength=N).astype(np.float32) + 1.0
        dinv = (1.0 / np.sqrt(deg)).astype(np.float32)
        dinvs.append(np.concatenate([dinv, np.zeros(NPAD - N, np.float32)]))
    structs = [build_structure(eis[t], ews[t], dinvs[t][:N]) for t in range(T)]

    # common per-section per-block L (max over cores; degree-sorted so tight)
    commonL = []
    for s in range(16):
        nb = max(st[s]["nblocks"] for st in structs)
        Lc = np.zeros(nb, np.int32)
        for st in structs:
            Ls = st[s]["Ls"]
            Lc[:len(Ls)] = np.maximum(Lc[:len(Ls)], Ls)
        commonL.append(Lc)

    tok_per_sec = [int(L.sum()) * P for L in commonL]
    blk_per_sec = [len(L) for L in commonL]
    tok_total = sum(tok_per_sec)

    per_gidx, per_w, per_sidx = [], [], []
    for c in range(T):
        g_all, w_all, s_all = [], [], []
        for s in range(16):
            sec = structs[c][s]
            Lc = commonL[s]
            nbm = len(Lc)
            own_nb = sec["nblocks"]
            # per-block dense grids in common shape
            for b in range(nbm):
                Lb = int(Lc[b])
                gi = np.zeros((Lb, P), np.int64)
                gw = np.zeros((Lb, P), np.float32)
                if b < own_nb:
                    sel = (sec["e_rank"] >= b * P) & (sec["e_rank"] < (b + 1) * P)
                    rr = sec["e_rank"][sel] - b * P
                    ss = sec["e_slot"][sel]
                    gi[ss, rr] = sec["e_src"][sel]
                    gw[ss, rr] = sec["e_w"][sel]
                g_all.append(gi.reshape(-1))
                w_all.append(gw.reshape(-1))
                if b < own_nb:
                    nd = sec["nodes"][b * P:(b + 1) * P].astype(np.int64)
                else:
                    nd = np.arange(QN, QN + P, dtype=np.int64)  # dummy block
                s_all.append(nd)
        per_gidx.append(np.concatenate(g_all).astype(np.int16))
        per_w.append(np.concatenate(w_all).astype(np.float32))
        per_sidx.append(np.concatenate(s_all).astype(np.int16))

    scat_total = sum(blk_per_sec) * P

    # blocked dinv layouts [128, NBLK]: col k = nodes k*128+p
    dinv_blk = [d.reshape(NBLK, P).T.copy() for d in dinvs]
    dinv2_blk = [(d * d).reshape(NBLK, P).T.copy() for d in dinvs]

    x_pad = np.zeros((NPAD, XD), np.float32)
    x_pad[:N] = x
    b1b = np.broadcast_to(b1, (P, H)).copy()
    b2b = np.broadcast_to(b2, (P, H)).copy()

    phase_times["host_layout"] = _time.time() - _t0
    _t0 = _time.time()

    # ---------------- K1 program ----------------
    nc1 = bacc.Bacc(trn_type="TRN2", num_devices=NCORES, num_swdge_queues=4)
    x_d = nc1.dram_tensor("x", [NPAD, XD], F32, kind="ExternalInput")
    W1_d = nc1.dram_tensor("W1", [XD, H], F32, kind="ExternalInput")
    W2_d = nc1.dram_tensor("W2", [H, H], F32, kind="ExternalInput")
    b1_d = nc1.dram_tensor("b1b", [P, H], F32, kind="ExternalInput")
    b2_d = nc1.dram_tensor("b2b", [P, H], F32, kind="ExternalInput")
    dinv_d = nc1.dram_tensor("dinv_blk", [P, NBLK], F32, kind="ExternalInput")
    dinv2_d = nc1.dram_tensor("dinv2_blk", [P, NBLK], F32, kind="ExternalInput")
    gidx_d = nc1.dram_tensor("gidx", [P, tok_total // 16], I16, kind="ExternalInput")
    gw_d = nc1.dram_tensor("gw", [P, tok_total // P], F32, kind="ExternalInput")
    sidx_d = nc1.dram_tensor("sidx", [P, scat_total // 16], I16, kind="ExternalInput")
    tanhE_d = nc1.dram_tensor("tanhE", [NPAD, H], F32, kind="ExternalOutput")

    table1 = nc1.dram_tensor("table1", [NPAD, H], F32)
    table2 = nc1.dram_tensor("table2", [NPAD, H], F32)
    acc = [nc1.dram_tensor(f"acc{pr}", [4 * ACCR, H], F32) for pr in range(2)]

    with tile.TileContext(nc1) as tc:
        with (
            tc.tile_pool(name="const", bufs=1) as cpool,
            tc.tile_pool(name="work", bufs=3) as wpool,
            tc.tile_pool(name="gath", bufs=3) as gpool,
            tc.tile_pool(name="psum", bufs=2, space="PSUM") as ppool,
        ):
            ident = cpool.tile([P, P], F32)
            make_identity(nc1, ident[:])
            W1_t = cpool.tile([XD, H], F32)
            W2_t = cpool.tile([H, H], F32)
            b1_t = cpool.tile([P, H], F32)
            b2_t = cpool.tile([P, H], F32)
            dinv_t = cpool.tile([P, NBLK], F32)
            dinv2_t = cpool.tile([P, NBLK], F32)
            nc1.sync.dma_start(out=W1_t[:], in_=W1_d[:])
            nc1.sync.dma_start(out=W2_t[:], in_=W2_d[:])
            nc1.sync.dma_start(out=b1_t[:], in_=b1_d[:])
            nc1.sync.dma_start(out=b2_t[:], in_=b2_d[:])
            nc1.sync.dma_start(out=dinv_t[:], in_=dinv_d[:])
            nc1.sync.dma_start(out=dinv2_t[:], in_=dinv2_d[:])

            # zero accumulators
            zt = cpool.tile([P, 512], F32)
            nc1.gpsimd.memset(zt[:], 0.0)
            for pr in range(2):
                rows = 4 * ACCR
                a0 = 0
                while a0 < rows:
                    a1 = min(a0 + 1024, rows)
                    nc1.sync.dma_start(out=acc[pr][a0:a1, :],
                                       in_=zt[:, :(a1 - a0) // 2])
                    a0 = a1

            # stage A: table1 = dinv * (x @ W1)
            for k in range(NBLK):
                xb = wpool.tile([P, XD], F32, tag="xb")
                nc1.sync.dma_start(out=xb[:], in_=x_d[k * P:(k + 1) * P, :])
                xT_p = ppool.tile([P, P], F32, tag="pt")
                nc1.tensor.transpose(out=xT_p[:], in_=xb[:], identity=ident[:])
                xT = wpool.tile([P, P], F32, tag="xT")
                nc1.vector.tensor_copy(out=xT[:], in_=xT_p[:])
                mm = ppool.tile([P, H], F32, tag="mm")
                nc1.tensor.matmul(out=mm[:], lhsT=xT[:], rhs=W1_t[:],
                                  start=True, stop=True)
                tb = wpool.tile([P, H], F32, tag="tb")
                nc1.vector.tensor_scalar_mul(
                    out=tb[:], in0=mm[:], scalar1=dinv_t[:, k:k + 1])
                nc1.sync.dma_start(out=table1[k * P:(k + 1) * P, :], in_=tb[:])

            # the two props
            for pr in range(2):
                table = table1 if pr == 0 else table2
                go = 0   # token offset
                so = 0   # scatter token offset
                for s in range(16):
                    r, q = divmod(s, 4)
                    Lc = commonL[s]
                    tbl_slice = table[q * QN:(q + 1) * QN, :]
                    b = 0
                    while b < len(Lc):
                        L = int(Lc[b])
                        b2_ = b
                        while b2_ < len(Lc) and int(Lc[b2_]) == L:
                            b2_ += 1
                        nb = b2_ - b
                        if L == 0:
                            b = b2_
                            continue
                        # chunk over blocks (and slots if L>64)
                        gpc = max(1, 64 // L) if L <= 64 else 1
                        sub = min(L, 64)
                        bb = b
                        while bb < b2_:
                            nbb = min(gpc, b2_ - bb)
                            if L <= 64:
                                ncols = nbb * L
                                tok = ncols * P
                                gt = gpool.tile([P, 64, H], F32, tag="g")
                                gi_t = gpool.tile([P, 512], I16, tag="gi")
                                w_t = gpool.tile([P, 64], F32, tag="gwt")
                                nc1.sync.dma_start(
                                    out=gi_t[:, :tok // 16],
                                    in_=gidx_d[:, go // 16:(go + tok) // 16])
                                nc1.sync.dma_start(
                                    out=w_t[:, :ncols],
                                    in_=gw_d[:, go // P:(go + tok) // P])
                                nc1.gpsimd.dma_gather(
                                    out_ap=gt[:, :ncols, :], in_ap=tbl_slice,
                                    idxs_ap=gi_t[:, :tok // 16],
                                    num_idxs=tok, num_idxs_reg=tok,
                                    elem_size=H, single_packet=False,
                                    queue_num=(bb + s) % 4)
                                nc1.vector.tensor_tensor(
                                    out=gt[:, :ncols, :], in0=gt[:, :ncols, :],
                                    in1=w_t[:, :ncols].to_broadcast([P, ncols, H]),
                                    op=mybir.AluOpType.mult)
                                # tree reduce per g-group: view [P, nbb, L, H]
                                gv = gt[:, :ncols, :].rearrange(
                                    "p (g l) h -> p g l h", l=L)
                                width = L
                                while width > 1:
                                    half = width // 2
                                    nc1.vector.tensor_tensor(
                                        out=gv[:, :, :half, :],
                                        in0=gv[:, :, :half, :],
                                        in1=gv[:, :, width - half:width, :],
                                        op=mybir.AluOpType.add)
                                    width = width - half
                                # pack partials [P, nbb, H]
                                pk = gpool.tile([P, 64, H], F32, tag="pk")
                                nc1.vector.tensor_copy(
                                    out=pk[:, :nbb, :], in_=gv[:, :, 0, :])
                                stok_all = nbb * P
                                si_t = gpool.tile([P, 512], I16, tag="si")
                                nc1.sync.dma_start(
                                    out=si_t[:, :stok_all // 16],
                                    in_=sidx_d[:, so // 16:(so + stok_all) // 16])
                                for c0 in range(0, nbb, 8):
                                    c1 = min(c0 + 8, nbb)
                                    stok = (c1 - c0) * P
                                    nc1.gpsimd.dma_scatter_add(
                                        acc[pr][r * ACCR:(r + 1) * ACCR, :],
                                        pk[:, c0:c1, :],
                                        si_t[:, c0 * 8:c0 * 8 + stok // 16],
                                        stok, stok, H)
                                so += stok_all
                                go += tok
                            else:
                                # L > 64: one block, slot sub-chunks
                                first = True
                                for s0 in range(0, L, sub):
                                    s1 = min(s0 + sub, L)
                                    ncols = s1 - s0
                                    tok = ncols * P
                                    gt = gpool.tile([P, 64, H], F32, tag="g")
                                    gi_t = gpool.tile([P, 512], I16, tag="gi")
                                    w_t = gpool.tile([P, 64], F32, tag="gwt")
                                    nc1.sync.dma_start(
                                        out=gi_t[:, :tok // 16],
                                        in_=gidx_d[:, go // 16:(go + tok) // 16])
                                    nc1.sync.dma_start(
                                        out=w_t[:, :ncols],
                                        in_=gw_d[:, go // P:(go + tok) // P])
                                    nc1.gpsimd.dma_gather(
                                        out_ap=gt[:, :ncols, :], in_ap=tbl_slice,
                                        idxs_ap=gi_t[:, :tok // 16],
                                        num_idxs=tok, num_idxs_reg=tok,
                                        elem_size=H, single_packet=False,
                                        queue_num=(bb + s0 + s) % 4)
                                    nc1.vector.tensor_tensor(
                                        out=gt[:, :ncols, :], in0=gt[:, :ncols, :],
                                        in1=w_t[:, :ncols].to_broadcast([P, ncols, H]),
                                        op=mybir.AluOpType.mult)
                                    width = ncols
                                    gv = gt[:, :ncols, :].rearrange(
                                        "p (g l) h -> p g l h", l=ncols)
                                    while width > 1:
                                        half = width // 2
                                        nc1.vector.tensor_tensor(
                                            out=gv[:, :, :half, :],
                                            in0=gv[:, :, :half, :],
                                            in1=gv[:, :, width - half:width, :],
                                            op=mybir.AluOpType.add)
                                        width = width - half
                                    pk = gpool.tile([P, 64, H], F32, tag="pk")
                                    nc1.vector.tensor_copy(
                                        out=pk[:, :1, :], in_=gv[:, :, 0, :])
                                    si_t = gpool.tile([P, 64], I16, tag="si")
                                    nc1.sync.dma_start(
                                        out=si_t[:, :P // 16],
                                        in_=sidx_d[:, so // 16:(so + P) // 16])
                                    nc1.gpsimd.dma_scatter_add(
                                        acc[pr][r * ACCR:(r + 1) * ACCR, :],
                                        pk[:, :1, :], si_t[:, :P // 16],
                                        P, P, H)
                                    go += tok
                                    first = False
                                so += P
                            bb += nbb if L <= 64 else 1
                        b = b2_

                # finish pass
                for k in range(NBLK):
                    r = (k * P) // QN
                    lrow = (k * P) % QN
                    ab = wpool.tile([P, H], F32, tag="ab")
                    nc1.sync.dma_start(
                        out=ab[:], in_=acc[pr][r * ACCR + lrow:r * ACCR + lrow + P, :])
                    tb = wpool.tile([P, H], F32, tag="tb2")
                    nc1.sync.dma_start(out=tb[:], in_=table[k * P:(k + 1) * P, :])
                    init = wpool.tile([P, H], F32, tag="init")
                    nc1.vector.tensor_scalar_mul(
                        out=init[:], in0=tb[:], scalar1=dinv_t[:, k:k + 1])
                    nc1.vector.tensor_tensor(out=ab[:], in0=ab[:], in1=init[:],
                                             op=mybir.AluOpType.add)
                    if pr == 0:
                        # h1 = relu(ab + b1); table2 = dinv * h1
                        nc1.vector.tensor_tensor(out=ab[:], in0=ab[:], in1=b1_t[:],
                                                 op=mybir.AluOpType.add)
                        nc1.vector.tensor_scalar_max(out=ab[:], in0=ab[:], scalar1=0.0)
                        ot = wpool.tile([P, H], F32, tag="ot")
                        nc1.vector.tensor_scalar_mul(
                            out=ot[:], in0=ab[:], scalar1=dinv_t[:, k:k + 1])
                        nc1.sync.dma_start(out=table2[k * P:(k + 1) * P, :], in_=ot[:])
                    else:
                        # embeds = ab @ W2 + b2 ; tanhE = tanh(embeds)
                        tp = ppool.tile([P, P], F32, tag="pt")
                        nc1.tensor.transpose(out=tp[:H, :], in_=ab[:],
                                             identity=ident[:])
                        abT = wpool.tile([P, P], F32, tag="abT")
                        nc1.vector.tensor_copy(out=abT[:H, :], in_=tp[:H, :])
                        mm = ppool.tile([P, H], F32, tag="mm")
                        nc1.tensor.matmul(out=mm[:], lhsT=abT[:H, :], rhs=W2_t[:],
                                          start=True, stop=True)
                        eb = wpool.tile([P, H], F32, tag="eb")
                        nc1.vector.tensor_tensor(out=eb[:], in0=mm[:], in1=b2_t[:],
                                                 op=mybir.AluOpType.add)
                        th = wpool.tile([P, H], F32, tag="th")
                        nc1.scalar.activation(
                            out=th[:], in_=eb[:],
                            func=mybir.ActivationFunctionType.Tanh)
                        nc1.sync.dma_start(out=tanhE_d[k * P:(k + 1) * P, :],
                                           in_=th[:])
    nc1.compile()
    phase_times["build_k1"] = _time.time() - _t0
    _t0 = _time.time()

    in_maps1 = []
    for c in range(NCORES):
        in_maps1.append({
            "x": x_pad, "W1": W1, "W2": W2, "b1b": b1b, "b2b": b2b,
            "dinv_blk": dinv_blk[c], "dinv2_blk": dinv2_blk[c],
            "gidx": wrap16(per_gidx[c]),
            "gw": per_w[c].reshape(-1, P).T.copy(),
            "sidx": wrap16(per_sidx[c]),
            "tanhE": np.zeros((NPAD, H), np.float32),
        })
    res1 = run_bass_kernel_spmd(nc1, in_maps1, core_ids=list(range(NCORES)))
    phase_times["run_k1"] = _time.time() - _t0
    _t0 = _time.time()
    tanhE = np.stack([res1.results[c]["tanhE"] for c in range(NCORES)])  # [T,NPAD,H]

    # ---------------- K2: GRU + head, node-sharded ----------------
    WihT = Wih.T.copy()    # [H, 3H]
    WhhT = Whh.T.copy()
    WlinT = Wlin.T.copy()  # [H, Z]
    bihb = np.broadcast_to(bih, (P, 3 * H)).copy()
    bhhb = np.broadcast_to(bhh, (P, 3 * H)).copy()
    blinb = np.broadcast_to(blin, (P, Z)).copy()

    nc2 = bacc.Bacc(trn_type="TRN2", num_devices=NCORES, num_swdge_queues=1)
    xs_d = nc2.dram_tensor("xs", [T, NSH, H], F32, kind="ExternalInput")
    WihT_d = nc2.dram_tensor("WihT", [H, 3 * H], F32, kind="ExternalInput")
    WhhT_d = nc2.dram_tensor("WhhT", [H, 3 * H], F32, kind="ExternalInput")
    WlinT_d = nc2.dram_tensor("WlinT", [H, Z], F32, kind="ExternalInput")
    bih_d = nc2.dram_tensor("bihb", [P, 3 * H], F32, kind="ExternalInput")
    bhh_d = nc2.dram_tensor("bhhb", [P, 3 * H], F32, kind="ExternalInput")
    blin_d = nc2.dram_tensor("blinb", [P, Z], F32, kind="ExternalInput")
    ys_d = nc2.dram_tensor("ys", [T, NSH, Z], F32, kind="ExternalOutput")

    with tile.TileContext(nc2) as tc:
        with (
            tc.tile_pool(name="const", bufs=1) as cpool,
            tc.tile_pool(name="state", bufs=1) as spool,
            tc.tile_pool(name="work", bufs=3) as wpool,
            tc.tile_pool(name="psum", bufs=2, space="PSUM") as ppool,
        ):
            ident = cpool.tile([P, P], F32)
            make_identity(nc2, ident[:])
            WihT_t = cpool.tile([H, 3 * H], F32)
            WhhT_t = cpool.tile([H, 3 * H], F32)
            WlinT_t = cpool.tile([H, Z], F32)
            bih_t = cpool.tile([P, 3 * H], F32)
            bhh_t = cpool.tile([P, 3 * H], F32)
            blin_t = cpool.tile([P, Z], F32)
            for tt, dd in ((WihT_t, WihT_d), (WhhT_t, WhhT_d), (WlinT_t, WlinT_d),
                           (bih_t, bih_d), (bhh_t, bhh_d), (blin_t, blin_d)):
                nc2.sync.dma_start(out=tt[:], in_=dd[:])

            JC2 = JC // 2
            NSH2 = NSH // 2
            for half in range(2):
                h_t = spool.tile([P, JC2, H], F32, tag="h")
                nc2.gpsimd.memset(h_t[:], 0.0)
                for t in range(T):
                    xs_t = spool.tile([P, JC2, H], F32, tag="xs")
                    xv = xs_d[t][half * NSH2:(half + 1) * NSH2, :].rearrange(
                        "(p j) h -> p j h", j=JC2)
                    nc2.sync.dma_start(out=xs_t[:], in_=xv)
                    gi_t = spool.tile([P, JC2, 3 * H], F32, tag="gi")
                    gh_t = spool.tile([P, JC2, 3 * H], F32, tag="gh")
                    for j in range(JC2):
                        for which in range(2):
                            srcT = xs_t if which == 0 else h_t
                            dstT = gi_t if which == 0 else gh_t
                            tp = ppool.tile([P, P], F32, tag="pt")
                            nc2.tensor.transpose(out=tp[:H, :], in_=srcT[:, j, :],
                                                 identity=ident[:])
                            sT = wpool.tile([H, P], F32, tag="sT")
                            nc2.vector.tensor_copy(out=sT[:], in_=tp[:H, :])
                            mm = ppool.tile([P, 3 * H], F32, tag="mm")
                            WT = WihT_t if which == 0 else WhhT_t
                            nc2.tensor.matmul(out=mm[:], lhsT=sT[:], rhs=WT[:],
                                              start=True, stop=True)
                            bT = bih_t if which == 0 else bhh_t
                            nc2.vector.tensor_tensor(out=dstT[:, j, :], in0=mm[:],
                                                     in1=bT[:],
                                                     op=mybir.AluOpType.add)
                    rz = spool.tile([P, JC2, 2 * H], F32, tag="rz")
                    nc2.vector.tensor_tensor(out=rz[:], in0=gi_t[:, :, :2 * H],
                                             in1=gh_t[:, :, :2 * H],
                                             op=mybir.AluOpType.add)
                    nc2.scalar.activation(out=rz[:], in_=rz[:],
                                          func=mybir.ActivationFunctionType.Sigmoid)
                    nn_t = spool.tile([P, JC2, H], F32, tag="nn")
                    nc2.vector.tensor_tensor(out=nn_t[:], in0=rz[:, :, :H],
                                             in1=gh_t[:, :, 2 * H:],
                                             op=mybir.AluOpType.mult)
                    nc2.vector.tensor_tensor(out=nn_t[:], in0=nn_t[:],
                                             in1=gi_t[:, :, 2 * H:],
                                             op=mybir.AluOpType.add)
                    nc2.scalar.activation(out=nn_t[:], in_=nn_t[:],
                                          func=mybir.ActivationFunctionType.Tanh)
                    dz = spool.tile([P, JC2, H], F32, tag="dz")
                    nc2.vector.tensor_tensor(out=dz[:], in0=h_t[:], in1=nn_t[:],
                                             op=mybir.AluOpType.subtract)
                    nc2.vector.tensor_tensor(out=dz[:], in0=dz[:], in1=rz[:, :, H:],
                                             op=mybir.AluOpType.mult)
                    nc2.vector.tensor_tensor(out=h_t[:], in0=nn_t[:], in1=dz[:],
                                             op=mybir.AluOpType.add)
                    ys_t = spool.tile([P, JC2, Z], F32, tag="ys")
                    for j in range(JC2):
                        tp = ppool.tile([P, P], F32, tag="pt")
                        nc2.tensor.transpose(out=tp[:H, :], in_=h_t[:, j, :],
                                             identity=ident[:])
                        sT = wpool.tile([H, P], F32, tag="sT")
                        nc2.vector.tensor_copy(out=sT[:], in_=tp[:H, :])
                        mm = ppool.tile([P, Z], F32, tag="mmz")
                        nc2.tensor.matmul(out=mm[:], lhsT=sT[:], rhs=WlinT_t[:],
                                          start=True, stop=True)
                        nc2.vector.tensor_tensor(out=ys_t[:, j, :], in0=mm[:],
                                                 in1=blin_t[:],
                                                 op=mybir.AluOpType.add)
                    yv = ys_d[t][half * NSH2:(half + 1) * NSH2, :].rearrange(
                        "(p j) z -> p j z", j=JC2)
                    nc2.sync.dma_start(out=yv, in_=ys_t[:])
    nc2.compile()
    phase_times["build_k2"] = _time.time() - _t0
    _t0 = _time.time()

    in_maps2 = []
    for c in range(NCORES):
        xs = np.ascontiguousarray(tanhE[:, c * NSH:(c + 1) * NSH, :])
        in_maps2.append({
            "xs": xs, "WihT": WihT, "WhhT": WhhT, "WlinT": WlinT,
            "bihb": bihb, "bhhb": bhhb, "blinb": blinb,
            "ys": np.zeros((T, NSH, Z), np.float32),
        })
    res2 = run_bass_kernel_spmd(nc2, in_maps2, core_ids=list(range(NCORES)))
    phase_times["run_k2"] = _time.time() - _t0
    out = np.concatenate([res2.results[c]["ys"] for c in range(NCORES)], axis=1)
    return np.ascontiguousarray(out[:, :N, :])

